# revision 62
# baseline (speedup 1.0000x reference)
"""GAT (2-layer, 4-head) Trainium2 kernel over 8 NeuronCores.

Strategy:
  * Edges sorted by dst, dst-range partitioned across the 8 cores (each core
    owns N/8 node rows and fully computes their output -> no output
    all-reduce, softmax stats stay core-local).
  * Per layer: node GEMM is data-parallel over the owned node range and also
    produces per-node s_dst scores (kept SBUF-resident); the 512B/row node
    table h is AllGathered in TWO halves (so the first half's edge gathers
    overlap the second collective); the edge phase gathers h[src] rows with
    dma_gather (each gather split over 4 SWDGE queues -- the gathers are
    per-descriptor-overhead-bound) and performs the segment softmax +
    weighted scatter-add as one-hot matmuls accumulated in PSUM.
  * Per-edge s_src = <h_gathered, a_src> is computed on DVE (multiply +
    reduce); per-edge s_dst is precomputed during the GEMM/AllGather window
    from a host-provided transposed one-hot (partition_broadcast + is_equal
    + tiny PE matmuls) into a 5KB SBUF table -- no per-edge score gather.
  * Hidden features are stored (d,h)-interleaved (col = d*4 + h) so the
    per-edge alpha scaling multiply has unit-stride innermost APs on every
    operand (DVE 2x/4x mode); all weights/BN constants are permuted
    host-side to match.
  * The edge phase is emitted software-pipelined in four stages offset by
    group (gather+one-hot | scores | scatter+normalize | BN tail) so no
    in-order engine queue head ever waits on the newest gather.
  * int16 gather indices cap at 32767: each AllGather half's flat table
    (8 cores x ~3100 rows) stays under the cap, with per-(node-tile) A/B
    edge classes by source half.
"""

import sys

if "/opt/trn_rl_repo" not in sys.path:
    sys.path.insert(0, "/opt/trn_rl_repo")

import ml_dtypes
import numpy as np

import concourse.bacc as bacc
import concourse.bass as bass
import concourse.mybir as mybir
import concourse.tile as tile
from concourse.bass_utils import run_bass_kernel_spmd

BF16 = mybir.dt.bfloat16
F32 = mybir.dt.float32
I16 = mybir.dt.int16
I32 = mybir.dt.int32

NCORES = 8
P = 128

CFG = dict(
    N=50000,
    E=500000,
    F=256,      # feature width (in = hid = 256)
    H=4,
    DH=64,
    OUT=64,
    ROW=256,    # bf16 row length of node table (512B)
    G=2,        # node tiles per gather group
    SPLITQ=1,   # split each gather across SWDGE queues
    NSWQ=4,
)


# --------------------------------------------------------------------------
# host-side preparation
# --------------------------------------------------------------------------

def _head_matrix(a):
    """[H, DH] -> block diagonal [F, H] so that s = h @ A."""
    H, DH = np.asarray(a).shape
    A = np.zeros((H * DH, H), np.float64)
    for h in range(H):
        A[h * DH:(h + 1) * DH, h] = np.asarray(a, np.float64)[h]
    return A


def _perm_n2o(F=256, H=4):
    """new column (d*H + h) <- old column (h*DH + d)."""
    DH = F // H
    n2o = np.empty(F, np.int64)
    for d in range(DH):
        for h in range(H):
            n2o[d * H + h] = h * DH + d
    return n2o


def _wfull(W, a_dst, perm_rows, n2o):
    """[W^T (cols (d,h)-interleaved) | W^T@Adst] as [F, 260] bf16.

    perm_rows: permute input-feature rows (layer>=2 sees permuted activations).
    """
    W = np.asarray(W, np.float64)
    Wt = W.T
    Bd = Wt @ _head_matrix(a_dst)
    out = np.zeros((W.shape[1], 260), np.float64)
    out[:, :W.shape[0]] = Wt[:, n2o]
    out[:, 256:260] = Bd
    if perm_rows:
        out = out[n2o, :]
    return out.astype(ml_dtypes.bfloat16)


def _bn_consts(gamma, beta, mean, var, n2o, eps=1e-5):
    gamma = np.asarray(gamma, np.float64)
    beta = np.asarray(beta, np.float64)
    mean = np.asarray(mean, np.float64)
    var = np.asarray(var, np.float64)
    g = (gamma / np.sqrt(var + eps))[n2o]
    b = (beta - mean * (gamma / np.sqrt(var + eps)))[n2o]
    F = gamma.shape[0]
    # [P, F//P]: col fc holds (permuted) features fc*128 .. fc*128+127
    return (
        np.ascontiguousarray(g.reshape(F // P, P).T.astype(np.float32)),
        np.ascontiguousarray(b.reshape(F // P, P).T.astype(np.float32)),
    )


def _wrap_idx(flat):
    """int16 position array -> dma_gather wrapped layout [128, len//16]."""
    n = len(flat)
    assert n % 16 == 0
    w = np.zeros((P, n // 16), np.int16)
    w[:16, :] = np.asarray(flat, np.int16).reshape(-1, 16).T
    w[16:, :] = np.tile(w[:16, :], (7, 1))
    return w


def prep_edges(cfg, edge_index):
    """Sort/partition edges; build per-core gather indices + dstrel tables.

    Edges are split into A/B halves by whether src falls in the first H1T
    tiles of its owner core (AllGather half 1) or the rest (half 2); each
    half's flat table [8 * half_rows] stays below the int16 index cap.
    """
    N, G = cfg["N"], cfg["G"]
    NB = ((N + NCORES - 1) // NCORES + P - 1) // P * P  # nodes per core (padded)
    NT = NB // P                                        # node tiles per core
    H1T = (NT + 1) // 2                                 # tiles in AG half 1
    H1R = H1T * P                                       # rows in AG half 1
    cfg["H1T"], cfg["H1R"] = H1T, H1R
    assert NCORES * H1R < 32768 and NCORES * (NB - H1R) < 32768
    src = np.asarray(edge_index[0], np.int64)
    dst = np.asarray(edge_index[1], np.int64)

    core = dst // NB
    tilein = (dst % NB) // P
    half = ((src % NB) >= H1R).astype(np.int64)
    order = np.lexsort((src, half, tilein, core))
    sc, tc, hc = core[order], tilein[order], half[order]
    ss, ds = src[order], dst[order]
    key = (sc * NT + tc) * 2 + hc
    bounds = np.searchsorted(key, np.arange(NCORES * NT * 2 + 1))
    lists = {}
    maxa = maxb = 1
    for k in range(NCORES):
        for t in range(NT):
            for h in (0, 1):
                j = (k * NT + t) * 2 + h
                i0, i1 = bounds[j], bounds[j + 1]
                lists[(k, t, h)] = (ss[i0:i1], ds[i0:i1] % P)
                if h == 0:
                    maxa = max(maxa, i1 - i0)
                else:
                    maxb = max(maxb, i1 - i0)
    KA = (maxa + P - 1) // P
    KB = (maxb + P - 1) // P
    K = KA + KB

    groups = []
    t0 = 0
    while t0 < NT:
        groups.append((t0, min(G, NT - t0)))
        t0 += G

    per_core = []
    for k in range(NCORES):
        idxa_cols, idxb_cols = [], []
        dstrel = np.full((P, NT * K), 128.0, np.float32)
        for (g0, gn) in groups:
            fa = np.zeros(gn * KA * P, np.int16)
            fb = np.zeros(gn * KB * P, np.int16)
            for tl in range(gn):
                t = g0 + tl
                for h in (0, 1):
                    s_arr, r_arr = lists[(k, t, h)]
                    n = len(s_arr)
                    sc, sr = s_arr // NB, s_arr % NB
                    if h == 0:
                        fa[tl * KA * P: tl * KA * P + n] = (
                            sc * H1R + sr).astype(np.int16)
                        col0 = g0 * K + tl * KA
                    else:
                        fb[tl * KB * P: tl * KB * P + n] = (
                            sc * (NB - H1R) + sr - H1R).astype(np.int16)
                        col0 = g0 * K + gn * KA + tl * KB
                    ii = np.arange(n)
                    dstrel[ii % P, col0 + ii // P] = r_arr
            idxa_cols.append(_wrap_idx(fa))
            idxb_cols.append(_wrap_idx(fb))
        # idxab: per group [A-cols | B-cols] concatenated
        ab_cols = []
        for ca, cb in zip(idxa_cols, idxb_cols):
            ab_cols.append(np.concatenate([ca, cb], axis=1))
        # dstrelT: flat [1, NT*K*P] with dstrelT[0, col*P + e] = dstrel[e, col]
        per_core.append(dict(
            idxab=np.concatenate(ab_cols, axis=1),
            dstrel=dstrel.astype(ml_dtypes.bfloat16),
            dstrelt=np.ascontiguousarray(
                dstrel.T.reshape(1, NT * K * P)).astype(ml_dtypes.bfloat16),
        ))
    return KA, KB, groups, per_core, NB, NT


# --------------------------------------------------------------------------
# device kernel
# --------------------------------------------------------------------------

def apx(base_ap, pairs, extra_offset=0):
    return bass.AP(base_ap.tensor, base_ap.offset + extra_offset,
                   [list(p) for p in pairs])


def build_kernel(cfg, KA, KB, groups, NB, NT):
    F, H, DH, OUT = cfg["F"], cfg["H"], cfg["DH"], cfg["OUT"]
    ROW = cfg["ROW"]
    H1T, H1R = cfg["H1T"], cfg["H1R"]
    H2T, H2R = NT - H1T, NB - H1R
    K = KA + KB
    FC = F // P
    AluOp = mybir.AluOpType
    Act = mybir.ActivationFunctionType

    nc = bacc.Bacc("TRN2", target_bir_lowering=False, debug=False,
                   num_devices=NCORES,
                   num_swdge_queues=cfg.get("NSWQ", 2))
    ABL = cfg.get("ABL", 5)

    # ---- I/O ----
    xt_in = nc.declare_dram_parameter("xt", [FC, P, NB], BF16, isOutput=False)
    wf_in = [nc.declare_dram_parameter(f"wfull{l + 1}", [FC, P, 260], BF16,
                                       isOutput=False) for l in range(2)]
    wct_in = nc.declare_dram_parameter("wct", [FC, P, OUT], BF16, isOutput=False)
    gv_in = [nc.declare_dram_parameter(f"gvec{l + 1}", [P, FC], F32,
                                       isOutput=False) for l in range(2)]
    bv_in = [nc.declare_dram_parameter(f"bvec{l + 1}", [P, FC], F32,
                                       isOutput=False) for l in range(2)]
    bc_in = nc.declare_dram_parameter("bc_rep", [P, OUT], F32, isOutput=False)
    SA = sum(gn * KA * 8 for _, gn in groups)
    SB = sum(gn * KB * 8 for _, gn in groups)
    idxab_in = nc.declare_dram_parameter("idxab", [P, SA + SB], I16,
                                         isOutput=False)
    ident_in = nc.declare_dram_parameter("ident", [P, P], F32, isOutput=False)
    iota_in = nc.declare_dram_parameter("iotarow", [P, P], BF16, isOutput=False)
    dstrel_in = nc.declare_dram_parameter("dstrel", [P, NT * K], BF16,
                                          isOutput=False)
    dstrelt_in = nc.declare_dram_parameter("dstrelt", [1, NT * K * P], BF16,
                                           isOutput=False)
    iotacol_in = nc.declare_dram_parameter("iotacol", [P, 1], BF16,
                                           isOutput=False)
    asrc_in = [nc.declare_dram_parameter(f"asrc{l + 1}", [P, F], BF16,
                                         isOutput=False) for l in range(2)]
    out_ext = nc.declare_dram_parameter("out", [NB, OUT], F32, isOutput=True)

    haug_own = [[nc.dram_tensor(f"haug_own{hf}_{l}", [H1R if hf == 0 else H2R,
                                                      ROW], BF16)
                 for hf in (0, 1)] for l in (0, 1)]
    haug_all = [[nc.dram_tensor(f"haug_all{hf}_{l}",
                                [NCORES, H1R if hf == 0 else H2R, ROW], BF16,
                                addr_space="Shared")
                 for hf in (0, 1)] for l in (0, 1)]

    with tile.TileContext(nc) as tc:
        with (
            tc.tile_pool(name="const", bufs=1) as cpool,
            tc.tile_pool(name="persist", bufs=1) as ppool,
            tc.tile_pool(name="work", bufs=3) as wpool,
            tc.tile_pool(name="edge", bufs=4) as epool,
            tc.tile_pool(name="dr", bufs=2) as dpool,
            tc.tile_pool(name="gath", bufs=4) as gpool,
            tc.tile_pool(name="hmul", bufs=2) as hpool,
            tc.tile_pool(name="znorm", bufs=4) as zpool,
            tc.tile_pool(name="psum", bufs=2, space="PSUM") as pspool,
            tc.tile_pool(name="psacc", bufs=4, space="PSUM") as accpool,
            tc.tile_pool(name="pstr", bufs=2, space="PSUM") as trpool,
        ):
            # ---- constants ----
            ident = cpool.tile([P, P], F32)
            nc.sync.dma_start(out=ident[:, :], in_=ident_in[:, :])
            identb = cpool.tile([P, P], BF16)
            nc.vector.tensor_copy(identb[:, :], ident[:, :])
            iota_bf = cpool.tile([P, P], BF16)
            nc.sync.dma_start(out=iota_bf[:, :], in_=iota_in[:, :])
            wf_sb = [cpool.tile([P, FC, 260], BF16, tag=f"wf{l}", name=f"wf{l}")
                     for l in range(2)]
            for l in range(2):
                nc.sync.dma_start(out=wf_sb[l][:, :, :],
                                  in_=wf_in[l].rearrange("c p n -> p c n"))
            wct_sb = cpool.tile([P, FC, OUT], BF16)
            nc.sync.dma_start(out=wct_sb[:, :, :],
                              in_=wct_in.rearrange("c p n -> p c n"))
            gv_sb = [cpool.tile([P, FC], F32, tag=f"gv{l}", name=f"gv{l}") for l in range(2)]
            bv_sb = [cpool.tile([P, FC], F32, tag=f"bv{l}", name=f"bv{l}") for l in range(2)]
            for l in range(2):
                nc.sync.dma_start(out=gv_sb[l][:, :], in_=gv_in[l][:, :])
                nc.sync.dma_start(out=bv_sb[l][:, :], in_=bv_in[l][:, :])
            bc_sb = cpool.tile([P, OUT], F32)
            nc.sync.dma_start(out=bc_sb[:, :], in_=bc_in[:, :])
            dstrel_sb = cpool.tile([P, NT * K], BF16)
            nc.sync.dma_start(out=dstrel_sb[:, :], in_=dstrel_in[:, :])
            idxab_sb = cpool.tile([P, SA + SB], I16)
            nc.sync.dma_start(out=idxab_sb[:, :], in_=idxab_in[:, :])
            iotacol = cpool.tile([P, 1], BF16)
            nc.sync.dma_start(out=iotacol[:, :], in_=iotacol_in[:, :])
            asrc_sb = [cpool.tile([P, F], BF16, tag=f"as{l}", name=f"as{l}")
                       for l in range(2)]
            for l in range(2):
                nc.sync.dma_start(out=asrc_sb[l][:, :], in_=asrc_in[l][:, :])

            # ---- activations (transposed, bf16, SBUF resident) ----
            xt_sb = [ppool.tile([P, FC, NB], BF16, tag=f"xt{l}", name=f"xt{l}")
                     for l in range(2)]
            nc.sync.dma_start(out=xt_sb[0][:, :, :],
                              in_=xt_in.rearrange("c p n -> p c n"))
            # per-layer per-node s_dst scores [P, NT, 4]
            sdst_sb = [ppool.tile([P, NT, H], BF16, tag=f"sd{l}", name=f"sd{l}")
                       for l in (0, 1)]
            # per-edge-slot s_dst, precomputed per layer [P, NT*K, H]
            sdpe_sb = ppool.tile([P, NT * K, H], BF16, tag="sdpe", name="sdpe")

            for rep_ in range(cfg.get("REPEAT", 1)):
                if ABL == 9:
                    nc.vector.memset(xt_sb[0][:, :, :], 0.1)
                for layer in (() if ABL == 9 else (0, 1)):
                    xt = xt_sb[layer % 2]
                    xtn = xt_sb[1 - layer % 2]
                    wfl = wf_sb[layer]

                    # ---- node GEMM -> haug_own + sdst (two AllGather halves;
                    # per-group s_dst precompute interleaved, gather-free) ----
                    def precompute_sdpe(gi):
                        g0, gn = groups[gi]
                        nslot = gn * K
                        tile_of = ([tl for tl in range(gn) for _ in range(KA)] +
                                   [tl for tl in range(gn) for _ in range(KB)])
                        offt = g0 * K * P
                        drt = dpool.tile([P, nslot * P], BF16, tag="drt")
                        nc.sync.dma_start(
                            out=drt[0:1, :],
                            in_=dstrelt_in[0:1, offt:offt + nslot * P])
                        nc.gpsimd.partition_broadcast(drt[:, :], drt[0:1, :])
                        ohT = dpool.tile([P, nslot, P], BF16, tag="ohT")
                        ic_ap = iotacol[:, :]
                        nc.vector.tensor_tensor(
                            out=ohT[:, :, :],
                            in0=apx(ic_ap, [ic_ap.ap[0], [0, nslot], [0, P]]),
                            in1=drt[:, :].rearrange("p (s e) -> p s e", s=nslot),
                            op=AluOp.is_equal)
                        sdp = accpool.tile([P, 260], F32, tag="acc")
                        for sl in range(nslot):
                            nc.tensor.matmul(
                                sdp[:, sl * H:(sl + 1) * H],
                                lhsT=ohT[:, sl, :],
                                rhs=sdst_sb[layer][:, g0 + tile_of[sl], :],
                                start=True, stop=True,
                            )
                        nc.scalar.copy(sdpe_sb[:, g0 * K:g0 * K + nslot, :],
                                       sdp[:, 0:nslot * H].rearrange(
                                           "p (s h) -> p s h", h=H))

                    gdone = 0
                    for hf, t0, t1 in ((0, 0, H1T), (1, H1T, NT)):
                        for t in range(t0, t1):
                            ps = pspool.tile([P, 260], F32, tag="gemm")
                            for kc in range(FC):
                                nc.tensor.matmul(
                                    ps[:, :],
                                    lhsT=xt[:, kc, t * P:(t + 1) * P],
                                    rhs=wfl[:, kc, :],
                                    start=(kc == 0), stop=(kc == FC - 1),
                                )
                            stg = wpool.tile([P, ROW], BF16, tag="gemmout")
                            nc.scalar.copy(stg[:, :], ps[:, 0:256])
                            nc.vector.tensor_copy(sdst_sb[layer][:, t, :],
                                                  ps[:, 256:260])
                            nc.sync.dma_start(
                                out=haug_own[layer][hf][(t - t0) * P:
                                                        (t - t0 + 1) * P, :],
                                in_=stg[:, :])
                            while (gdone < len(groups) and
                                   groups[gdone][0] + groups[gdone][1] <= t + 1):
                                if ABL not in (1, 4, 9, 10):
                                    precompute_sdpe(gdone)
                                gdone += 1
                        if ABL != 4:
                            nc.gpsimd.collective_compute(
                                "AllGather", AluOp.bypass,
                                replica_groups=[list(range(NCORES))],
                                ins=[haug_own[layer][hf][:, :]],
                                outs=[haug_all[layer][hf][:, :, :]],
                            )

                    # ---- share node table ----
                    if ABL == 4:
                        nc.vector.memset(xtn[:, :, :], 0.1)
                        continue
                    hflat1 = haug_all[layer][0].rearrange("c n d -> (c n) d")
                    hflat2 = haug_all[layer][1].rearrange("c n d -> (c n) d")
                    if ABL == 1:
                        nc.vector.memset(xtn[:, :, :], 0.1)
                        continue
                    if ABL == 10:
                        nc.vector.memset(sdpe_sb[:, :, :], 0.0)

                    # ---- edge phase: software-pipelined emission ----
                    # Stages offset by group so no engine queue head ever
                    # waits on the just-issued gather: A=gather+one-hot,
                    # B=per-edge scores, C=weighted scatter+normalize,
                    # D=transpose+BN+ELU.
                    state = {}
                    offab = 0
                    goffs = []
                    for (g0, gn) in groups:
                        goffs.append(offab)
                        offab += gn * (KA + KB) * 8

                    def stage_a(gi):
                        g0, gn = groups[gi]
                        nslot = gn * K
                        offab = goffs[gi]
                        ia = idxab_sb[:, offab:offab + gn * KA * 8]
                        ib = idxab_sb[:, offab + gn * KA * 8:
                                      offab + gn * (KA + KB) * 8]
                        oh = epool.tile([P, nslot, P], BF16, tag="oh")
                        dr = dstrel_sb[:, g0 * K:g0 * K + nslot]
                        iota_ap = iota_bf[:, :]
                        nc.vector.tensor_tensor(
                            out=oh[:, :, :],
                            in0=apx(iota_ap, [iota_ap.ap[0], [0, nslot], [1, P]]),
                            in1=dr.to_broadcast([P, nslot, P]),
                            op=AluOp.is_equal)
                        gat = gpool.tile([P, nslot, ROW], BF16, tag="gat")
                        npa, npb = gn * KA * P, gn * KB * P
                        if ABL == 2:
                            nc.vector.memset(gat[:, :, :], 0.05)
                            nc.vector.tensor_copy(gat[:, 0:1, 0:8], ia[:, 0:8])
                            nc.vector.tensor_copy(gat[:, 1:2, 0:8], ib[:, 0:8])
                        elif cfg.get("SPLITQ"):
                            ha, hb = gn * KA // 2, gn * KB // 2
                            for q, (s0, s1, tab, idx, i0) in enumerate((
                                (0, ha, hflat1, ia, 0),
                                (ha, gn * KA, hflat1, ia, ha),
                                (gn * KA, gn * KA + hb, hflat2, ib, 0),
                                (gn * KA + hb, nslot, hflat2, ib, hb),
                            )):
                                nsl = s1 - s0
                                if nsl == 0:
                                    continue
                                nc.gpsimd.dma_gather(
                                    out_ap=gat[:, s0:s1, :], in_ap=tab[:, :],
                                    idxs_ap=idx[:, i0 * 8:(i0 + nsl) * 8],
                                    num_idxs=nsl * P, num_idxs_reg=nsl * P,
                                    elem_size=ROW, single_packet=False,
                                    queue_num=q)
                        else:
                            nc.gpsimd.dma_gather(
                                out_ap=gat[:, 0:gn * KA, :],
                                in_ap=hflat1[:, :],
                                idxs_ap=ia, num_idxs=npa, num_idxs_reg=npa,
                                elem_size=ROW, single_packet=False)
                            nc.gpsimd.dma_gather(
                                out_ap=gat[:, gn * KA:nslot, :],
                                in_ap=hflat2[:, :],
                                idxs_ap=ib, num_idxs=npb, num_idxs_reg=npb,
                                elem_size=ROW, single_packet=False,
                                queue_num=1)
                        state[gi] = dict(oh=oh, gat=gat)

                    def stage_b(gi):
                        g0, gn = groups[gi]
                        nslot = gn * K
                        st = state[gi]
                        gat = st["gat"]
                        # s_src[e] = sum_d h[e, h*64+d] * a_src[h, d]
                        hts = hpool.tile([P, nslot, 260], BF16, tag="hts")
                        asrc_ap = asrc_sb[layer][:, :]
                        ssrc = wpool.tile([P, nslot, H], F32, tag="ssrc")
                        nc.vector.tensor_tensor(
                            out=hts[:, :, 0:256],
                            in0=gat[:, :, :],
                            in1=apx(asrc_ap, [asrc_ap.ap[0], [0, nslot], [1, F]]),
                            op=AluOp.mult)
                        nc.vector.tensor_reduce(
                            out=ssrc[:, :, :],
                            in_=hts[:, :, 0:256].rearrange(
                                "p s (d h) -> p s h d", h=H),
                            axis=mybir.AxisListType.X, op=AluOp.add)
                        # e = lrelu(ssrc + sdst); w = exp(e)
                        ef = wpool.tile([P, nslot, H], F32, tag="ef")
                        nc.vector.tensor_tensor(
                            out=ef[:, :, :], in0=ssrc[:, :, :],
                            in1=sdpe_sb[:, g0 * K:g0 * K + nslot, :],
                            op=AluOp.add)
                        nc.vector.scalar_tensor_tensor(
                            out=ef[:, :, :], in0=ef[:, :, :], scalar=0.2,
                            in1=ef[:, :, :], op0=AluOp.mult, op1=AluOp.max)
                        wexp = wpool.tile([P, nslot, H], BF16, tag="wexp")
                        nc.scalar.activation(wexp[:, :, :], ef[:, :, :], Act.Exp)
                        st.update(hts=hts, wexp=wexp)

                    def stage_c(gi):
                        g0, gn = groups[gi]
                        nslot = gn * K
                        st = state[gi]
                        gat, hts, wexp, oh = (st["gat"], st["hts"], st["wexp"],
                                              st["oh"])
                        # scale gathered rows by w; ones-cols = w itself
                        # (d,h)-interleaved feature order: wexp broadcast has
                        # unit-stride innermost -> DVE fast mode
                        wexp_ap = wexp[:, :, :]
                        nc.vector.tensor_tensor(
                            out=hts[:, :, 0:256].rearrange(
                                "p s (d h) -> p s d h", h=H),
                            in0=gat[:, :, :].rearrange("p s (d h) -> p s d h", h=H),
                            in1=apx(wexp_ap, [wexp_ap.ap[0], wexp_ap.ap[1],
                                              [0, DH], [1, H]]),
                            op=AluOp.mult)
                        nc.vector.tensor_copy(hts[:, :, 256:260], wexp[:, :, :])
                        accs, zsbs = [], []
                        for tl in range(gn):
                            acc = accpool.tile([P, 260], F32, tag="acc")
                            slots = ([tl * KA + s for s in range(KA)] +
                                     [gn * KA + tl * KB + s for s in range(KB)])
                            for j, sl in enumerate(slots):
                                nc.tensor.matmul(
                                    acc[:, :],
                                    lhsT=oh[:, sl, :],
                                    rhs=hts[:, sl, :],
                                    start=(j == 0), stop=(j == len(slots) - 1),
                                )
                            # normalize by T = acc[:, 256:260]
                            tmax = wpool.tile([P, H], F32, tag="tmax")
                            nc.vector.tensor_scalar(
                                out=tmax[:, :], in0=acc[:, 256:260], scalar1=1e-9,
                                scalar2=None, op0=AluOp.max)
                            rec = wpool.tile([P, H], F32, tag="rec")
                            nc.vector.reciprocal(rec[:, :], tmax[:, :])
                            zsb = zpool.tile([P, F], BF16, tag="zsb")
                            rec_ap = rec[:, :]
                            nc.vector.tensor_tensor(
                                out=zsb[:, :].rearrange("p (d h) -> p d h", h=H),
                                in0=acc[:, 0:256].rearrange("p (d h) -> p d h", h=H),
                                in1=apx(rec_ap, [rec_ap.ap[0], [0, DH], [1, H]]),
                                op=AluOp.mult)
                            zsbs.append(zsb)
                        st["zsbs"] = zsbs

                    def stage_d(gi):
                        g0, gn = groups[gi]
                        st = state.pop(gi)
                        for tl in range(gn):
                            t = g0 + tl
                            zsb = st["zsbs"][tl]
                            # transpose + BN + ELU per feature chunk
                            for fc in range(FC):
                                pst = trpool.tile([P, P], BF16, tag="tr")
                                nc.tensor.transpose(
                                    pst[:, :], zsb[:, fc * P:(fc + 1) * P],
                                    identb[:, :])
                                # both Act ops read pst (parallel, not chained)
                                ybn = wpool.tile([P, P], F32, tag="ybn")
                                nc.scalar.activation(
                                    ybn[:, :], pst[:, :], Act.Identity,
                                    bias=bv_sb[layer][:, fc:fc + 1],
                                    scale=gv_sb[layer][:, fc:fc + 1])
                                ey = wpool.tile([P, P], F32, tag="ey")
                                nc.scalar.activation(
                                    ey[:, :], pst[:, :], Act.Exp,
                                    bias=bv_sb[layer][:, fc:fc + 1],
                                    scale=gv_sb[layer][:, fc:fc + 1])
                                # elu(y) = min(exp(y)-1, 0) + max(y, 0)
                                nc.vector.tensor_scalar(
                                    out=ey[:, :], in0=ey[:, :], scalar1=1.0,
                                    scalar2=0.0, op0=AluOp.subtract, op1=AluOp.min)
                                nc.vector.scalar_tensor_tensor(
                                    out=xtn[:, fc, t * P:(t + 1) * P],
                                    in0=ybn[:, :], scalar=0.0, in1=ey[:, :],
                                    op0=AluOp.max, op1=AluOp.add)

                    ngrp = len(groups)
                    do_b = ABL not in (20,)
                    do_c = ABL not in (20, 21)
                    do_d = ABL not in (20, 21, 22)
                    if not do_d:
                        nc.vector.memset(xtn[:, :, :], 0.1)
                    for i in range(ngrp + 3):
                        if i < ngrp:
                            stage_a(i)
                        if do_b and 1 <= i < ngrp + 1:
                            stage_b(i - 1)
                        if do_c and 2 <= i < ngrp + 2:
                            stage_c(i - 2)
                        if do_d and 3 <= i:
                            stage_d(i - 3)
                        elif not do_d and 3 <= i:
                            state.pop(i - 3, None)

                # ---- classifier ----
                for t in range(NT):
                    ps = pspool.tile([P, 260], F32, tag="gemm")
                    for kc in range(FC):
                        nc.tensor.matmul(
                            ps[:, 0:OUT],
                            lhsT=xt_sb[0][:, kc, t * P:(t + 1) * P],
                            rhs=wct_sb[:, kc, :],
                            start=(kc == 0), stop=(kc == FC - 1),
                        )
                    ob = wpool.tile([P, OUT], F32, tag="ob")
                    nc.vector.tensor_tensor(out=ob[:, :], in0=ps[:, 0:OUT],
                                            in1=bc_sb[:, :], op=AluOp.add)
                    nc.sync.dma_start(out=out_ext[t * P:(t + 1) * P, :],
                                      in_=ob[:, :])

    nc.compile()
    return nc


# --------------------------------------------------------------------------
# entry point
# --------------------------------------------------------------------------

def kernel(x, edge_index, W1, a_src1, a_dst1, bn1_gamma, bn1_beta, bn1_mean,
           bn1_var, W2, a_src2, a_dst2, bn2_gamma, bn2_beta, bn2_mean, bn2_var,
           Wc, bc, _cfg=None, _run_kwargs=None, _bench=0):
    cfg = dict(CFG)
    if _cfg:
        cfg.update(_cfg)
    N, F, OUT = cfg["N"], cfg["F"], cfg["OUT"]
    FC = F // P

    KA, KB, groups, per_core, NB, NT = prep_edges(cfg, edge_index)
    nc = build_kernel(cfg, KA, KB, groups, NB, NT)

    n2o = _perm_n2o(F, CFG["H"])
    wfull1 = _wfull(W1, a_dst1, False, n2o)
    wfull2 = _wfull(W2, a_dst2, True, n2o)
    wct = np.ascontiguousarray(
        np.asarray(Wc, np.float64).T[n2o, :]).astype(ml_dtypes.bfloat16)
    g1, b1 = _bn_consts(bn1_gamma, bn1_beta, bn1_mean, bn1_var, n2o)
    g2, b2 = _bn_consts(bn2_gamma, bn2_beta, bn2_mean, bn2_var, n2o)
    bc_rep = np.tile(np.asarray(bc, np.float32)[None, :], (P, 1))
    asrc1 = np.tile(np.asarray(a_src1, np.float32).reshape(1, F)[:, n2o],
                    (P, 1)).astype(ml_dtypes.bfloat16)
    asrc2 = np.tile(np.asarray(a_src2, np.float32).reshape(1, F)[:, n2o],
                    (P, 1)).astype(ml_dtypes.bfloat16)

    xpad = np.zeros((NB * NCORES, F), np.float32)
    xpad[:N] = np.asarray(x, np.float32)
    xt = np.ascontiguousarray(xpad.T).astype(ml_dtypes.bfloat16)  # [F, NPAD]

    in_maps = []
    for k in range(NCORES):
        xk = xt[:, k * NB:(k + 1) * NB].reshape(FC, P, NB)
        in_maps.append(dict(
            xt=np.ascontiguousarray(xk),
            wfull1=np.ascontiguousarray(wfull1.reshape(FC, P, 260)),
            wfull2=np.ascontiguousarray(wfull2.reshape(FC, P, 260)),
            wct=np.ascontiguousarray(wct.reshape(FC, P, OUT)),
            gvec1=g1, bvec1=b1, gvec2=g2, bvec2=b2, bc_rep=bc_rep,
            asrc1=asrc1, asrc2=asrc2,
            idxab=per_core[k]["idxab"],
            dstrel=per_core[k]["dstrel"],
            dstrelt=per_core[k]["dstrelt"],
            ident=np.eye(P, dtype=np.float32),
            iotarow=np.tile(np.arange(P, dtype=np.float32)[None, :],
                            (P, 1)).astype(ml_dtypes.bfloat16),
            iotacol=np.arange(P, dtype=np.float32).reshape(P, 1).astype(
                ml_dtypes.bfloat16),
        ))

    res = run_bass_kernel_spmd(nc, in_maps, list(range(NCORES)),
                               **(_run_kwargs or {}))
    out = np.concatenate([res.results[k]["out"] for k in range(NCORES)], axis=0)
    out = out[:N].astype(np.float32)
    if _bench:
        steady, legacy = _bench_pjrt(nc, in_maps, _bench)
        return out, (steady, legacy)
    if _run_kwargs is not None:
        return out, res
    return out


def _bench_pjrt(nc, in_maps, iters):
    """Median per-iteration wall time (ns) of the NEFF execution via PJRT,
    device-resident inputs, back-to-back async dispatch."""
    import time
    import jax
    import jax.numpy as jnp
    from jax.sharding import Mesh, PartitionSpec
    from jax.experimental.shard_map import shard_map
    from concourse import bass2jax
    from concourse.bass2jax import _bass_exec_p, partition_id_tensor
    import concourse.mybir as mybir

    n_cores = len(in_maps)
    partition_name = nc.partition_id_tensor.name if nc.partition_id_tensor else None
    in_names, out_names, out_avals, zero_outs = [], [], [], []
    for alloc in nc.m.functions[0].allocations:
        if not isinstance(alloc, mybir.MemoryLocationSet):
            continue
        name = alloc.memorylocations[0].name
        if alloc.kind == "ExternalInput":
            if name != partition_name:
                in_names.append(name)
        elif alloc.kind == "ExternalOutput":
            shape = list(alloc.tensor_shape)
            dt = mybir.dt.np(alloc.dtype)
            out_avals.append(jax.core.ShapedArray(shape, dt))
            out_names.append(name)
            zero_outs.append(np.zeros(shape, dt))
    n_params = len(in_names)
    n_outs = len(out_names)
    in_names.extend(out_names)
    if partition_name is not None:
        in_names.append(partition_name)
    donate = tuple(range(n_params, n_params + n_outs))

    def _body(*args):
        operands = list(args)
        if partition_name is not None:
            operands.append(partition_id_tensor())
        return tuple(_bass_exec_p.bind(
            *operands, out_avals=tuple(out_avals), in_names=tuple(in_names),
            out_names=tuple(out_names), lowering_input_output_aliases=(),
            sim_require_finite=True, sim_require_nnan=True, nc=nc))

    devices = jax.devices()[:n_cores]
    mesh = Mesh(np.asarray(devices), ("core",))
    sharded = jax.jit(
        shard_map(_body, mesh=mesh,
                  in_specs=(PartitionSpec("core"),) * (n_params + n_outs),
                  out_specs=(PartitionSpec("core"),) * n_outs,
                  check_rep=False),
        donate_argnums=(), keep_unused=True)
    per_core = [[np.asarray(m[name]) for name in in_names[:n_params]]
                for m in in_maps]
    concat_in = [np.concatenate([per_core[c][i] for c in range(n_cores)], axis=0)
                 for i in range(n_params)]
    from jax.sharding import NamedSharding
    sh = NamedSharding(mesh, PartitionSpec("core"))
    dev_in = [jax.device_put(a, sh) for a in concat_in]
    zshapes = [(n_cores * z.shape[0], *z.shape[1:]) for z in zero_outs]
    zdtypes = [z.dtype for z in zero_outs]

    dev_zeros = [jax.device_put(np.zeros(s_, d_), sh)
                 for s_, d_ in zip(zshapes, zdtypes)]

    def one_iter():
        return sharded(*dev_in, *dev_zeros)

    def streak(n):
        t0 = time.perf_counter()
        outs = [one_iter() for _ in range(n)]
        jax.block_until_ready(outs[-1])
        return (time.perf_counter() - t0) * 1e9

    jax.block_until_ready(one_iter())
    n_lo, n_hi = iters, iters + 100
    t_lo = min(streak(n_lo) for _ in range(4))
    t_hi = min(streak(n_hi) for _ in range(4))
    # Marginal per-iteration time over the extra 100 pipelined iterations:
    # subtracts the one-time dispatch-pipeline fill (~70ms/streak on axon)
    # that both streaks pay, leaving steady-state per-iteration HW time.
    steady = (t_hi - t_lo) / (n_hi - n_lo)
    legacy = t_lo / n_lo
    return steady, legacy


# revision 63
# speedup vs baseline: 1.1131x; 1.1131x over previous
"""GAT (2-layer, 4-head) Trainium2 kernel over 8 NeuronCores.

Strategy:
  * Edges sorted by dst, dst-range partitioned across the 8 cores (each core
    owns N/8 node rows and fully computes their output -> no output
    all-reduce, softmax stats stay core-local).
  * Per layer: node GEMM is data-parallel over the owned node range and also
    produces per-node s_dst scores (kept SBUF-resident); the 512B/row node
    table h is AllGathered in TWO halves (so the first half's edge gathers
    overlap the second collective); the edge phase gathers h[src] rows with
    dma_gather (each gather split over 4 SWDGE queues -- the gathers are
    per-descriptor-overhead-bound) and performs the segment softmax +
    weighted scatter-add as one-hot matmuls accumulated in PSUM.
  * Per-edge s_src = <h_gathered, a_src> is computed on DVE (multiply +
    reduce); per-edge s_dst is precomputed during the GEMM/AllGather window
    from a host-provided transposed one-hot (partition_broadcast + is_equal
    + tiny PE matmuls) into a 5KB SBUF table -- no per-edge score gather.
  * Hidden features are stored (d,h)-interleaved (col = d*4 + h) so the
    per-edge alpha scaling multiply has unit-stride innermost APs on every
    operand (DVE 2x/4x mode); all weights/BN constants are permuted
    host-side to match.
  * The edge phase is emitted software-pipelined in four stages offset by
    group (gather+one-hot | scores | scatter+normalize | BN tail) so no
    in-order engine queue head ever waits on the newest gather.
  * int16 gather indices cap at 32767: each AllGather half's flat table
    (8 cores x ~3100 rows) stays under the cap, with per-(node-tile) A/B
    edge classes by source half.
"""

import sys

if "/opt/trn_rl_repo" not in sys.path:
    sys.path.insert(0, "/opt/trn_rl_repo")

import ml_dtypes
import numpy as np

import concourse.bacc as bacc
import concourse.bass as bass
import concourse.mybir as mybir
import concourse.tile as tile
from concourse.bass_utils import run_bass_kernel_spmd

BF16 = mybir.dt.bfloat16
F32 = mybir.dt.float32
I16 = mybir.dt.int16
I32 = mybir.dt.int32

NCORES = 8
P = 128

CFG = dict(
    N=50000,
    E=500000,
    F=256,      # feature width (in = hid = 256)
    H=4,
    DH=64,
    OUT=64,
    ROW=256,    # bf16 row length of node table (512B)
    G=2,        # node tiles per gather group
    SPLITQ=1,   # split each gather across SWDGE queues
    NSWQ=4,
)


# --------------------------------------------------------------------------
# host-side preparation
# --------------------------------------------------------------------------

def _head_matrix(a):
    """[H, DH] -> block diagonal [F, H] so that s = h @ A."""
    H, DH = np.asarray(a).shape
    A = np.zeros((H * DH, H), np.float64)
    for h in range(H):
        A[h * DH:(h + 1) * DH, h] = np.asarray(a, np.float64)[h]
    return A


def _perm_n2o(F=256, H=4):
    """new column (d*H + h) <- old column (h*DH + d)."""
    DH = F // H
    n2o = np.empty(F, np.int64)
    for d in range(DH):
        for h in range(H):
            n2o[d * H + h] = h * DH + d
    return n2o


def _wfull(W, a_dst, perm_rows, n2o):
    """[W^T (cols (d,h)-interleaved) | W^T@Adst] as [F, 260] bf16.

    perm_rows: permute input-feature rows (layer>=2 sees permuted activations).
    """
    W = np.asarray(W, np.float64)
    Wt = W.T
    Bd = Wt @ _head_matrix(a_dst)
    out = np.zeros((W.shape[1], 260), np.float64)
    out[:, :W.shape[0]] = Wt[:, n2o]
    out[:, 256:260] = Bd
    if perm_rows:
        out = out[n2o, :]
    return out.astype(ml_dtypes.bfloat16)


def _bn_consts(gamma, beta, mean, var, n2o, eps=1e-5):
    gamma = np.asarray(gamma, np.float64)
    beta = np.asarray(beta, np.float64)
    mean = np.asarray(mean, np.float64)
    var = np.asarray(var, np.float64)
    g = (gamma / np.sqrt(var + eps))[n2o]
    b = (beta - mean * (gamma / np.sqrt(var + eps)))[n2o]
    F = gamma.shape[0]
    # [P, F//P]: col fc holds (permuted) features fc*128 .. fc*128+127
    return (
        np.ascontiguousarray(g.reshape(F // P, P).T.astype(np.float32)),
        np.ascontiguousarray(b.reshape(F // P, P).T.astype(np.float32)),
    )


def _wrap_idx(flat):
    """int16 position array -> dma_gather wrapped layout [128, len//16]."""
    n = len(flat)
    assert n % 16 == 0
    w = np.zeros((P, n // 16), np.int16)
    w[:16, :] = np.asarray(flat, np.int16).reshape(-1, 16).T
    w[16:, :] = np.tile(w[:16, :], (7, 1))
    return w


def prep_edges(cfg, edge_index):
    """Sort/partition edges; build per-core gather indices + dstrel tables.

    Edges are split into A/B halves by whether src falls in the first H1T
    tiles of its owner core (AllGather half 1) or the rest (half 2); each
    half's flat table [8 * half_rows] stays below the int16 index cap.
    """
    N, G = cfg["N"], cfg["G"]
    NB = ((N + NCORES - 1) // NCORES + P - 1) // P * P  # nodes per core (padded)
    NT = NB // P                                        # node tiles per core
    H1T = (NT + 1) // 2                                 # tiles in AG half 1
    H1R = H1T * P                                       # rows in AG half 1
    cfg["H1T"], cfg["H1R"] = H1T, H1R
    assert NCORES * H1R < 32768 and NCORES * (NB - H1R) < 32768
    src = np.asarray(edge_index[0], np.int64)
    dst = np.asarray(edge_index[1], np.int64)

    core = dst // NB
    tilein = (dst % NB) // P
    half = ((src % NB) >= H1R).astype(np.int64)
    order = np.lexsort((src, half, tilein, core))
    sc, tc, hc = core[order], tilein[order], half[order]
    ss, ds = src[order], dst[order]
    key = (sc * NT + tc) * 2 + hc
    bounds = np.searchsorted(key, np.arange(NCORES * NT * 2 + 1))
    lists = {}
    maxa = maxb = 1
    for k in range(NCORES):
        for t in range(NT):
            for h in (0, 1):
                j = (k * NT + t) * 2 + h
                i0, i1 = bounds[j], bounds[j + 1]
                lists[(k, t, h)] = (ss[i0:i1], ds[i0:i1] % P)
                if h == 0:
                    maxa = max(maxa, i1 - i0)
                else:
                    maxb = max(maxb, i1 - i0)
    KA = (maxa + P - 1) // P
    KB = (maxb + P - 1) // P
    K = KA + KB

    groups = []
    t0 = 0
    while t0 < NT:
        groups.append((t0, min(G, NT - t0)))
        t0 += G

    per_core = []
    for k in range(NCORES):
        idxa_cols, idxb_cols = [], []
        dstrel = np.full((P, NT * K), 128.0, np.float32)
        for (g0, gn) in groups:
            fa = np.zeros(gn * KA * P, np.int16)
            fb = np.zeros(gn * KB * P, np.int16)
            for tl in range(gn):
                t = g0 + tl
                for h in (0, 1):
                    s_arr, r_arr = lists[(k, t, h)]
                    n = len(s_arr)
                    sc, sr = s_arr // NB, s_arr % NB
                    if h == 0:
                        fa[tl * KA * P: tl * KA * P + n] = (
                            sc * H1R + sr).astype(np.int16)
                        col0 = g0 * K + tl * KA
                    else:
                        fb[tl * KB * P: tl * KB * P + n] = (
                            sc * (NB - H1R) + sr - H1R).astype(np.int16)
                        col0 = g0 * K + gn * KA + tl * KB
                    ii = np.arange(n)
                    dstrel[ii % P, col0 + ii // P] = r_arr
            idxa_cols.append(_wrap_idx(fa))
            idxb_cols.append(_wrap_idx(fb))
        # idxab: per group [A-cols | B-cols] concatenated
        ab_cols = []
        for ca, cb in zip(idxa_cols, idxb_cols):
            ab_cols.append(np.concatenate([ca, cb], axis=1))
        # dstrelT: flat [1, NT*K*P] with dstrelT[0, col*P + e] = dstrel[e, col]
        per_core.append(dict(
            idxab=np.concatenate(ab_cols, axis=1),
            dstrel=dstrel.astype(ml_dtypes.bfloat16),
            dstrelt=np.ascontiguousarray(
                dstrel.T.reshape(1, NT * K * P)).astype(ml_dtypes.bfloat16),
        ))
    return KA, KB, groups, per_core, NB, NT


# --------------------------------------------------------------------------
# device kernel
# --------------------------------------------------------------------------

def apx(base_ap, pairs, extra_offset=0):
    return bass.AP(base_ap.tensor, base_ap.offset + extra_offset,
                   [list(p) for p in pairs])


def build_kernel(cfg, KA, KB, groups, NB, NT):
    F, H, DH, OUT = cfg["F"], cfg["H"], cfg["DH"], cfg["OUT"]
    ROW = cfg["ROW"]
    H1T, H1R = cfg["H1T"], cfg["H1R"]
    H2T, H2R = NT - H1T, NB - H1R
    K = KA + KB
    FC = F // P
    AluOp = mybir.AluOpType
    Act = mybir.ActivationFunctionType

    nc = bacc.Bacc("TRN2", target_bir_lowering=False, debug=False,
                   num_devices=NCORES,
                   num_swdge_queues=cfg.get("NSWQ", 2))
    ABL = cfg.get("ABL", 5)

    # ---- I/O ----
    xt_in = nc.declare_dram_parameter("xt", [FC, P, NB], BF16, isOutput=False)
    wf_in = [nc.declare_dram_parameter(f"wfull{l + 1}", [FC, P, 260], BF16,
                                       isOutput=False) for l in range(2)]
    wct_in = nc.declare_dram_parameter("wct", [FC, P, OUT], BF16, isOutput=False)
    gv_in = [nc.declare_dram_parameter(f"gvec{l + 1}", [P, FC], F32,
                                       isOutput=False) for l in range(2)]
    bv_in = [nc.declare_dram_parameter(f"bvec{l + 1}", [P, FC], F32,
                                       isOutput=False) for l in range(2)]
    bc_in = nc.declare_dram_parameter("bc_rep", [P, OUT], F32, isOutput=False)
    SA = sum(gn * KA * 8 for _, gn in groups)
    SB = sum(gn * KB * 8 for _, gn in groups)
    idxab_in = nc.declare_dram_parameter("idxab", [P, SA + SB], I16,
                                         isOutput=False)
    ident_in = nc.declare_dram_parameter("ident", [P, P], F32, isOutput=False)
    iota_in = nc.declare_dram_parameter("iotarow", [P, P], BF16, isOutput=False)
    dstrel_in = nc.declare_dram_parameter("dstrel", [P, NT * K], BF16,
                                          isOutput=False)
    dstrelt_in = nc.declare_dram_parameter("dstrelt", [1, NT * K * P], BF16,
                                           isOutput=False)
    iotacol_in = nc.declare_dram_parameter("iotacol", [P, 1], BF16,
                                           isOutput=False)
    asrc_in = [nc.declare_dram_parameter(f"asrc{l + 1}", [P, F], BF16,
                                         isOutput=False) for l in range(2)]
    out_ext = nc.declare_dram_parameter("out", [NB, OUT], F32, isOutput=True)

    haug_own = [[nc.dram_tensor(f"haug_own{hf}_{l}", [H1R if hf == 0 else H2R,
                                                      ROW], BF16)
                 for hf in (0, 1)] for l in (0, 1)]
    haug_all = [[nc.dram_tensor(f"haug_all{hf}_{l}",
                                [NCORES, H1R if hf == 0 else H2R, ROW], BF16,
                                addr_space="Shared")
                 for hf in (0, 1)] for l in (0, 1)]

    with tile.TileContext(nc) as tc:
        with (
            tc.tile_pool(name="const", bufs=1) as cpool,
            tc.tile_pool(name="persist", bufs=1) as ppool,
            tc.tile_pool(name="work", bufs=3) as wpool,
            tc.tile_pool(name="edge", bufs=3) as epool,
            tc.tile_pool(name="dr", bufs=2) as dpool,
            tc.tile_pool(name="gath", bufs=3) as gpool,
            tc.tile_pool(name="hmul", bufs=3) as hpool,
            tc.tile_pool(name="znorm", bufs=4) as zpool,
            tc.tile_pool(name="psum", bufs=2, space="PSUM") as pspool,
            tc.tile_pool(name="psacc", bufs=4, space="PSUM") as accpool,
            tc.tile_pool(name="pstr", bufs=2, space="PSUM") as trpool,
        ):
            # ---- constants ----
            ident = cpool.tile([P, P], F32)
            nc.sync.dma_start(out=ident[:, :], in_=ident_in[:, :])
            identb = cpool.tile([P, P], BF16)
            nc.vector.tensor_copy(identb[:, :], ident[:, :])
            iota_bf = cpool.tile([P, P], BF16)
            nc.sync.dma_start(out=iota_bf[:, :], in_=iota_in[:, :])
            wf_sb = [cpool.tile([P, FC, 260], BF16, tag=f"wf{l}", name=f"wf{l}")
                     for l in range(2)]
            for l in range(2):
                nc.sync.dma_start(out=wf_sb[l][:, :, :],
                                  in_=wf_in[l].rearrange("c p n -> p c n"))
            wct_sb = cpool.tile([P, FC, OUT], BF16)
            nc.sync.dma_start(out=wct_sb[:, :, :],
                              in_=wct_in.rearrange("c p n -> p c n"))
            gv_sb = [cpool.tile([P, FC], F32, tag=f"gv{l}", name=f"gv{l}") for l in range(2)]
            bv_sb = [cpool.tile([P, FC], F32, tag=f"bv{l}", name=f"bv{l}") for l in range(2)]
            for l in range(2):
                nc.sync.dma_start(out=gv_sb[l][:, :], in_=gv_in[l][:, :])
                nc.sync.dma_start(out=bv_sb[l][:, :], in_=bv_in[l][:, :])
            bc_sb = cpool.tile([P, OUT], F32)
            nc.sync.dma_start(out=bc_sb[:, :], in_=bc_in[:, :])
            dstrel_sb = cpool.tile([P, NT * K], BF16)
            nc.sync.dma_start(out=dstrel_sb[:, :], in_=dstrel_in[:, :])
            idxab_sb = cpool.tile([P, SA + SB], I16)
            nc.sync.dma_start(out=idxab_sb[:, :], in_=idxab_in[:, :])
            iotacol = cpool.tile([P, 1], BF16)
            nc.sync.dma_start(out=iotacol[:, :], in_=iotacol_in[:, :])
            asrc_sb = [cpool.tile([P, F], BF16, tag=f"as{l}", name=f"as{l}")
                       for l in range(2)]
            for l in range(2):
                nc.sync.dma_start(out=asrc_sb[l][:, :], in_=asrc_in[l][:, :])

            # ---- activations (transposed, bf16, SBUF resident) ----
            xt_sb = [ppool.tile([P, FC, NB], BF16, tag=f"xt{l}", name=f"xt{l}")
                     for l in range(2)]
            nc.sync.dma_start(out=xt_sb[0][:, :, :],
                              in_=xt_in.rearrange("c p n -> p c n"))
            # per-layer per-node s_dst scores [P, NT, 4]
            sdst_sb = [ppool.tile([P, NT, H], BF16, tag=f"sd{l}", name=f"sd{l}")
                       for l in (0, 1)]
            # per-edge-slot s_dst, precomputed per layer [P, NT*K, H]
            sdpe_sb = ppool.tile([P, NT * K, H], BF16, tag="sdpe", name="sdpe")

            for rep_ in range(cfg.get("REPEAT", 1)):
                if ABL == 9:
                    nc.vector.memset(xt_sb[0][:, :, :], 0.1)
                for layer in (() if ABL == 9 else (0, 1)):
                    xt = xt_sb[layer % 2]
                    xtn = xt_sb[1 - layer % 2]
                    wfl = wf_sb[layer]

                    # ---- node GEMM -> haug_own + sdst (two AllGather halves;
                    # per-group s_dst precompute interleaved, gather-free) ----
                    def precompute_sdpe(gi):
                        g0, gn = groups[gi]
                        nslot = gn * K
                        tile_of = ([tl for tl in range(gn) for _ in range(KA)] +
                                   [tl for tl in range(gn) for _ in range(KB)])
                        offt = g0 * K * P
                        drt = dpool.tile([P, nslot * P], BF16, tag="drt")
                        nc.sync.dma_start(
                            out=drt[0:1, :],
                            in_=dstrelt_in[0:1, offt:offt + nslot * P])
                        nc.gpsimd.partition_broadcast(drt[:, :], drt[0:1, :])
                        ohT = dpool.tile([P, nslot, P], BF16, tag="ohT")
                        ic_ap = iotacol[:, :]
                        nc.vector.tensor_tensor(
                            out=ohT[:, :, :],
                            in0=apx(ic_ap, [ic_ap.ap[0], [0, nslot], [0, P]]),
                            in1=drt[:, :].rearrange("p (s e) -> p s e", s=nslot),
                            op=AluOp.is_equal)
                        sdp = accpool.tile([P, 260], F32, tag="acc")
                        for sl in range(nslot):
                            nc.tensor.matmul(
                                sdp[:, sl * H:(sl + 1) * H],
                                lhsT=ohT[:, sl, :],
                                rhs=sdst_sb[layer][:, g0 + tile_of[sl], :],
                                start=True, stop=True,
                            )
                        nc.scalar.copy(sdpe_sb[:, g0 * K:g0 * K + nslot, :],
                                       sdp[:, 0:nslot * H].rearrange(
                                           "p (s h) -> p s h", h=H))

                    gdone = 0
                    for hf, t0, t1 in ((0, 0, H1T), (1, H1T, NT)):
                        for t in range(t0, t1):
                            ps = pspool.tile([P, 260], F32, tag="gemm")
                            for kc in range(FC):
                                nc.tensor.matmul(
                                    ps[:, :],
                                    lhsT=xt[:, kc, t * P:(t + 1) * P],
                                    rhs=wfl[:, kc, :],
                                    start=(kc == 0), stop=(kc == FC - 1),
                                )
                            stg = wpool.tile([P, ROW], BF16, tag="gemmout")
                            nc.scalar.copy(stg[:, :], ps[:, 0:256])
                            nc.vector.tensor_copy(sdst_sb[layer][:, t, :],
                                                  ps[:, 256:260])
                            nc.sync.dma_start(
                                out=haug_own[layer][hf][(t - t0) * P:
                                                        (t - t0 + 1) * P, :],
                                in_=stg[:, :])
                            while (gdone < len(groups) and
                                   groups[gdone][0] + groups[gdone][1] <= t + 1):
                                if ABL not in (1, 4, 9, 10):
                                    precompute_sdpe(gdone)
                                gdone += 1
                        if ABL != 4:
                            nc.gpsimd.collective_compute(
                                "AllGather", AluOp.bypass,
                                replica_groups=[list(range(NCORES))],
                                ins=[haug_own[layer][hf][:, :]],
                                outs=[haug_all[layer][hf][:, :, :]],
                            )

                    # ---- share node table ----
                    if ABL == 4:
                        nc.vector.memset(xtn[:, :, :], 0.1)
                        continue
                    hflat1 = haug_all[layer][0].rearrange("c n d -> (c n) d")
                    hflat2 = haug_all[layer][1].rearrange("c n d -> (c n) d")
                    if ABL == 1:
                        nc.vector.memset(xtn[:, :, :], 0.1)
                        continue
                    if ABL == 10:
                        nc.vector.memset(sdpe_sb[:, :, :], 0.0)

                    # ---- edge phase: software-pipelined emission ----
                    # Stages offset by group so no engine queue head ever
                    # waits on the just-issued gather: A=gather+one-hot,
                    # B=per-edge scores, C=weighted scatter+normalize,
                    # D=transpose+BN+ELU.
                    state = {}
                    offab = 0
                    goffs = []
                    for (g0, gn) in groups:
                        goffs.append(offab)
                        offab += gn * (KA + KB) * 8

                    def stage_a(gi):
                        g0, gn = groups[gi]
                        nslot = gn * K
                        offab = goffs[gi]
                        ia = idxab_sb[:, offab:offab + gn * KA * 8]
                        ib = idxab_sb[:, offab + gn * KA * 8:
                                      offab + gn * (KA + KB) * 8]
                        oh = epool.tile([P, nslot, P], BF16, tag="oh")
                        dr = dstrel_sb[:, g0 * K:g0 * K + nslot]
                        iota_ap = iota_bf[:, :]
                        nc.vector.tensor_tensor(
                            out=oh[:, :, :],
                            in0=apx(iota_ap, [iota_ap.ap[0], [0, nslot], [1, P]]),
                            in1=dr.to_broadcast([P, nslot, P]),
                            op=AluOp.is_equal)
                        gat = gpool.tile([P, nslot, ROW], BF16, tag="gat")
                        npa, npb = gn * KA * P, gn * KB * P
                        if ABL == 2:
                            nc.vector.memset(gat[:, :, :], 0.05)
                            nc.vector.tensor_copy(gat[:, 0:1, 0:8], ia[:, 0:8])
                            nc.vector.tensor_copy(gat[:, 1:2, 0:8], ib[:, 0:8])
                        elif cfg.get("SPLITQ"):
                            ha, hb = gn * KA // 2, gn * KB // 2
                            for q, (s0, s1, tab, idx, i0) in enumerate((
                                (0, ha, hflat1, ia, 0),
                                (ha, gn * KA, hflat1, ia, ha),
                                (gn * KA, gn * KA + hb, hflat2, ib, 0),
                                (gn * KA + hb, nslot, hflat2, ib, hb),
                            )):
                                nsl = s1 - s0
                                if nsl == 0:
                                    continue
                                nc.gpsimd.dma_gather(
                                    out_ap=gat[:, s0:s1, :], in_ap=tab[:, :],
                                    idxs_ap=idx[:, i0 * 8:(i0 + nsl) * 8],
                                    num_idxs=nsl * P, num_idxs_reg=nsl * P,
                                    elem_size=ROW, single_packet=False,
                                    queue_num=q)
                        else:
                            nc.gpsimd.dma_gather(
                                out_ap=gat[:, 0:gn * KA, :],
                                in_ap=hflat1[:, :],
                                idxs_ap=ia, num_idxs=npa, num_idxs_reg=npa,
                                elem_size=ROW, single_packet=False)
                            nc.gpsimd.dma_gather(
                                out_ap=gat[:, gn * KA:nslot, :],
                                in_ap=hflat2[:, :],
                                idxs_ap=ib, num_idxs=npb, num_idxs_reg=npb,
                                elem_size=ROW, single_packet=False,
                                queue_num=1)
                        state[gi] = dict(oh=oh, gat=gat)

                    def stage_b(gi):
                        g0, gn = groups[gi]
                        nslot = gn * K
                        st = state[gi]
                        gat = st["gat"]
                        # s_src[e] = sum_d h[e, h*64+d] * a_src[h, d]
                        hts = hpool.tile([P, nslot, 260], BF16, tag="hts")
                        asrc_ap = asrc_sb[layer][:, :]
                        ssrc = wpool.tile([P, nslot, H], F32, tag="ssrc")
                        nc.vector.tensor_tensor(
                            out=hts[:, :, 0:256],
                            in0=gat[:, :, :],
                            in1=apx(asrc_ap, [asrc_ap.ap[0], [0, nslot], [1, F]]),
                            op=AluOp.mult)
                        nc.vector.tensor_reduce(
                            out=ssrc[:, :, :],
                            in_=hts[:, :, 0:256].rearrange(
                                "p s (d h) -> p s h d", h=H),
                            axis=mybir.AxisListType.X, op=AluOp.add)
                        # e = lrelu(ssrc + sdst); w = exp(e)
                        ef = wpool.tile([P, nslot, H], F32, tag="ef")
                        nc.vector.tensor_tensor(
                            out=ef[:, :, :], in0=ssrc[:, :, :],
                            in1=sdpe_sb[:, g0 * K:g0 * K + nslot, :],
                            op=AluOp.add)
                        nc.vector.scalar_tensor_tensor(
                            out=ef[:, :, :], in0=ef[:, :, :], scalar=0.2,
                            in1=ef[:, :, :], op0=AluOp.mult, op1=AluOp.max)
                        wexp = wpool.tile([P, nslot, H], BF16, tag="wexp")
                        nc.scalar.activation(wexp[:, :, :], ef[:, :, :], Act.Exp)
                        st.update(hts=hts, wexp=wexp)

                    def stage_c(gi):
                        g0, gn = groups[gi]
                        nslot = gn * K
                        st = state[gi]
                        gat, hts, wexp, oh = (st["gat"], st["hts"], st["wexp"],
                                              st["oh"])
                        # scale gathered rows by w; ones-cols = w itself
                        # (d,h)-interleaved feature order: wexp broadcast has
                        # unit-stride innermost -> DVE fast mode
                        wexp_ap = wexp[:, :, :]
                        nc.vector.tensor_tensor(
                            out=hts[:, :, 0:256].rearrange(
                                "p s (d h) -> p s d h", h=H),
                            in0=gat[:, :, :].rearrange("p s (d h) -> p s d h", h=H),
                            in1=apx(wexp_ap, [wexp_ap.ap[0], wexp_ap.ap[1],
                                              [0, DH], [1, H]]),
                            op=AluOp.mult)
                        nc.vector.tensor_copy(hts[:, :, 256:260], wexp[:, :, :])
                        accs, zsbs = [], []
                        for tl in range(gn):
                            acc = accpool.tile([P, 260], F32, tag="acc")
                            slots = ([tl * KA + s for s in range(KA)] +
                                     [gn * KA + tl * KB + s for s in range(KB)])
                            for j, sl in enumerate(slots):
                                nc.tensor.matmul(
                                    acc[:, :],
                                    lhsT=oh[:, sl, :],
                                    rhs=hts[:, sl, :],
                                    start=(j == 0), stop=(j == len(slots) - 1),
                                )
                            # normalize by T = acc[:, 256:260]
                            tmax = wpool.tile([P, H], F32, tag="tmax")
                            nc.vector.tensor_scalar(
                                out=tmax[:, :], in0=acc[:, 256:260], scalar1=1e-9,
                                scalar2=None, op0=AluOp.max)
                            rec = wpool.tile([P, H], F32, tag="rec")
                            nc.vector.reciprocal(rec[:, :], tmax[:, :])
                            zsb = zpool.tile([P, F], BF16, tag="zsb")
                            rec_ap = rec[:, :]
                            nc.vector.tensor_tensor(
                                out=zsb[:, :].rearrange("p (d h) -> p d h", h=H),
                                in0=acc[:, 0:256].rearrange("p (d h) -> p d h", h=H),
                                in1=apx(rec_ap, [rec_ap.ap[0], [0, DH], [1, H]]),
                                op=AluOp.mult)
                            zsbs.append(zsb)
                        st["zsbs"] = zsbs

                    def stage_d(gi):
                        g0, gn = groups[gi]
                        st = state.pop(gi)
                        for tl in range(gn):
                            t = g0 + tl
                            zsb = st["zsbs"][tl]
                            # transpose + BN + ELU per feature chunk
                            for fc in range(FC):
                                pst = trpool.tile([P, P], BF16, tag="tr")
                                nc.tensor.transpose(
                                    pst[:, :], zsb[:, fc * P:(fc + 1) * P],
                                    identb[:, :])
                                # both Act ops read pst (parallel, not chained)
                                ybn = wpool.tile([P, P], F32, tag="ybn")
                                nc.scalar.activation(
                                    ybn[:, :], pst[:, :], Act.Identity,
                                    bias=bv_sb[layer][:, fc:fc + 1],
                                    scale=gv_sb[layer][:, fc:fc + 1])
                                ey = wpool.tile([P, P], F32, tag="ey")
                                nc.scalar.activation(
                                    ey[:, :], pst[:, :], Act.Exp,
                                    bias=bv_sb[layer][:, fc:fc + 1],
                                    scale=gv_sb[layer][:, fc:fc + 1])
                                # elu(y) = min(exp(y)-1, 0) + max(y, 0)
                                nc.vector.tensor_scalar(
                                    out=ey[:, :], in0=ey[:, :], scalar1=1.0,
                                    scalar2=0.0, op0=AluOp.subtract, op1=AluOp.min)
                                nc.vector.scalar_tensor_tensor(
                                    out=xtn[:, fc, t * P:(t + 1) * P],
                                    in0=ybn[:, :], scalar=0.0, in1=ey[:, :],
                                    op0=AluOp.max, op1=AluOp.add)

                    ngrp = len(groups)
                    do_b = ABL not in (20,)
                    do_c = ABL not in (20, 21)
                    do_d = ABL not in (20, 21, 22)
                    if not do_d:
                        nc.vector.memset(xtn[:, :, :], 0.1)
                    for i in range(ngrp + 3):
                        if i < ngrp:
                            stage_a(i)
                        if do_b and 1 <= i < ngrp + 1:
                            stage_b(i - 1)
                        if do_c and 2 <= i < ngrp + 2:
                            stage_c(i - 2)
                        if do_d and 3 <= i:
                            stage_d(i - 3)
                        elif not do_d and 3 <= i:
                            state.pop(i - 3, None)

                # ---- classifier ----
                for t in range(NT):
                    ps = pspool.tile([P, 260], F32, tag="gemm")
                    for kc in range(FC):
                        nc.tensor.matmul(
                            ps[:, 0:OUT],
                            lhsT=xt_sb[0][:, kc, t * P:(t + 1) * P],
                            rhs=wct_sb[:, kc, :],
                            start=(kc == 0), stop=(kc == FC - 1),
                        )
                    ob = wpool.tile([P, OUT], F32, tag="ob")
                    nc.vector.tensor_tensor(out=ob[:, :], in0=ps[:, 0:OUT],
                                            in1=bc_sb[:, :], op=AluOp.add)
                    nc.sync.dma_start(out=out_ext[t * P:(t + 1) * P, :],
                                      in_=ob[:, :])

    nc.compile()
    return nc


# --------------------------------------------------------------------------
# entry point
# --------------------------------------------------------------------------

def kernel(x, edge_index, W1, a_src1, a_dst1, bn1_gamma, bn1_beta, bn1_mean,
           bn1_var, W2, a_src2, a_dst2, bn2_gamma, bn2_beta, bn2_mean, bn2_var,
           Wc, bc, _cfg=None, _run_kwargs=None, _bench=0):
    cfg = dict(CFG)
    if _cfg:
        cfg.update(_cfg)
    N, F, OUT = cfg["N"], cfg["F"], cfg["OUT"]
    FC = F // P

    KA, KB, groups, per_core, NB, NT = prep_edges(cfg, edge_index)
    nc = build_kernel(cfg, KA, KB, groups, NB, NT)

    n2o = _perm_n2o(F, CFG["H"])
    wfull1 = _wfull(W1, a_dst1, False, n2o)
    wfull2 = _wfull(W2, a_dst2, True, n2o)
    wct = np.ascontiguousarray(
        np.asarray(Wc, np.float64).T[n2o, :]).astype(ml_dtypes.bfloat16)
    g1, b1 = _bn_consts(bn1_gamma, bn1_beta, bn1_mean, bn1_var, n2o)
    g2, b2 = _bn_consts(bn2_gamma, bn2_beta, bn2_mean, bn2_var, n2o)
    bc_rep = np.tile(np.asarray(bc, np.float32)[None, :], (P, 1))
    asrc1 = np.tile(np.asarray(a_src1, np.float32).reshape(1, F)[:, n2o],
                    (P, 1)).astype(ml_dtypes.bfloat16)
    asrc2 = np.tile(np.asarray(a_src2, np.float32).reshape(1, F)[:, n2o],
                    (P, 1)).astype(ml_dtypes.bfloat16)

    xpad = np.zeros((NB * NCORES, F), np.float32)
    xpad[:N] = np.asarray(x, np.float32)
    xt = np.ascontiguousarray(xpad.T).astype(ml_dtypes.bfloat16)  # [F, NPAD]

    in_maps = []
    for k in range(NCORES):
        xk = xt[:, k * NB:(k + 1) * NB].reshape(FC, P, NB)
        in_maps.append(dict(
            xt=np.ascontiguousarray(xk),
            wfull1=np.ascontiguousarray(wfull1.reshape(FC, P, 260)),
            wfull2=np.ascontiguousarray(wfull2.reshape(FC, P, 260)),
            wct=np.ascontiguousarray(wct.reshape(FC, P, OUT)),
            gvec1=g1, bvec1=b1, gvec2=g2, bvec2=b2, bc_rep=bc_rep,
            asrc1=asrc1, asrc2=asrc2,
            idxab=per_core[k]["idxab"],
            dstrel=per_core[k]["dstrel"],
            dstrelt=per_core[k]["dstrelt"],
            ident=np.eye(P, dtype=np.float32),
            iotarow=np.tile(np.arange(P, dtype=np.float32)[None, :],
                            (P, 1)).astype(ml_dtypes.bfloat16),
            iotacol=np.arange(P, dtype=np.float32).reshape(P, 1).astype(
                ml_dtypes.bfloat16),
        ))

    res = run_bass_kernel_spmd(nc, in_maps, list(range(NCORES)),
                               **(_run_kwargs or {}))
    out = np.concatenate([res.results[k]["out"] for k in range(NCORES)], axis=0)
    out = out[:N].astype(np.float32)
    if _bench:
        steady, legacy = _bench_pjrt(nc, in_maps, _bench)
        return out, (steady, legacy)
    if _run_kwargs is not None:
        return out, res
    return out


def _bench_pjrt(nc, in_maps, iters):
    """Median per-iteration wall time (ns) of the NEFF execution via PJRT,
    device-resident inputs, back-to-back async dispatch."""
    import time
    import jax
    import jax.numpy as jnp
    from jax.sharding import Mesh, PartitionSpec
    from jax.experimental.shard_map import shard_map
    from concourse import bass2jax
    from concourse.bass2jax import _bass_exec_p, partition_id_tensor
    import concourse.mybir as mybir

    n_cores = len(in_maps)
    partition_name = nc.partition_id_tensor.name if nc.partition_id_tensor else None
    in_names, out_names, out_avals, zero_outs = [], [], [], []
    for alloc in nc.m.functions[0].allocations:
        if not isinstance(alloc, mybir.MemoryLocationSet):
            continue
        name = alloc.memorylocations[0].name
        if alloc.kind == "ExternalInput":
            if name != partition_name:
                in_names.append(name)
        elif alloc.kind == "ExternalOutput":
            shape = list(alloc.tensor_shape)
            dt = mybir.dt.np(alloc.dtype)
            out_avals.append(jax.core.ShapedArray(shape, dt))
            out_names.append(name)
            zero_outs.append(np.zeros(shape, dt))
    n_params = len(in_names)
    n_outs = len(out_names)
    in_names.extend(out_names)
    if partition_name is not None:
        in_names.append(partition_name)
    donate = tuple(range(n_params, n_params + n_outs))

    def _body(*args):
        operands = list(args)
        if partition_name is not None:
            operands.append(partition_id_tensor())
        return tuple(_bass_exec_p.bind(
            *operands, out_avals=tuple(out_avals), in_names=tuple(in_names),
            out_names=tuple(out_names), lowering_input_output_aliases=(),
            sim_require_finite=True, sim_require_nnan=True, nc=nc))

    devices = jax.devices()[:n_cores]
    mesh = Mesh(np.asarray(devices), ("core",))
    sharded = jax.jit(
        shard_map(_body, mesh=mesh,
                  in_specs=(PartitionSpec("core"),) * (n_params + n_outs),
                  out_specs=(PartitionSpec("core"),) * n_outs,
                  check_rep=False),
        donate_argnums=(), keep_unused=True)
    per_core = [[np.asarray(m[name]) for name in in_names[:n_params]]
                for m in in_maps]
    concat_in = [np.concatenate([per_core[c][i] for c in range(n_cores)], axis=0)
                 for i in range(n_params)]
    from jax.sharding import NamedSharding
    sh = NamedSharding(mesh, PartitionSpec("core"))
    dev_in = [jax.device_put(a, sh) for a in concat_in]
    zshapes = [(n_cores * z.shape[0], *z.shape[1:]) for z in zero_outs]
    zdtypes = [z.dtype for z in zero_outs]

    dev_zeros = [jax.device_put(np.zeros(s_, d_), sh)
                 for s_, d_ in zip(zshapes, zdtypes)]

    def one_iter():
        return sharded(*dev_in, *dev_zeros)

    def streak(n):
        t0 = time.perf_counter()
        outs = [one_iter() for _ in range(n)]
        jax.block_until_ready(outs[-1])
        return (time.perf_counter() - t0) * 1e9

    jax.block_until_ready(one_iter())
    n_lo, n_hi = iters, iters + 100
    t_lo = min(streak(n_lo) for _ in range(4))
    t_hi = min(streak(n_hi) for _ in range(4))
    # Marginal per-iteration time over the extra 100 pipelined iterations:
    # subtracts the one-time dispatch-pipeline fill (~70ms/streak on axon)
    # that both streaks pay, leaving steady-state per-iteration HW time.
    steady = (t_hi - t_lo) / (n_hi - n_lo)
    legacy = t_lo / n_lo
    return steady, legacy
